# revision 39
# baseline (speedup 1.0000x reference)
"""Causal self-attention on 8 TRN2 NeuronCores.

Sharding: core c = (batch b = c // 2, head-group g = c % 2).
Each core handles one batch and 8 of the 16 heads:
  - QKV projection for its 512 q/k/v feature slices (transposed layout)
  - causal attention for its 8 heads
  - partial output projection (its 512 rows of W_out)
Host sums the two partials per batch and adds b_out.

All TensorE matmuls run in bf16; softmax runs in f32 (exp on ScalarE,
normalization via M=1 ones-matmul denominators + VectorE reciprocal).

Key structure:
  - Scores matmuls have K=64 contraction; even/odd heads of a pair sit at
    SBUF partitions 0-63 / 64-127 and run concurrently in the top/bottom
    halves of the PE array (row tiling).
  - Causal band: for the diagonal key chunks only the live query suffix
    (width W = 512-128*l) is computed, exp'd and AV'd.
  - AV is column-tiled: the two heads' V tiles sit in array columns 0-63 /
    64-127 and stream their probability tiles concurrently (2x vs M=65).
  - Softmax denominators are M=1 ones-stationary matmuls into four PSUM
    partitions (0/32/64/96 by key-chunk parity), also running column-tiled.
"""

import numpy as np
import ml_dtypes

B, T, D, H = 4, 2048, 1024, 16
HG = 2            # head groups (tensor-parallel factor)
HL = H // HG      # 8 heads per core
HD = D // H       # 64
DG = HL * HD      # 512 features per group
SCALE = 1.0 / float(np.sqrt(HD))
NCORES = 8
TCH = T // 128    # 16 key chunks of 128
NQC = T // 512    # 4 query chunks of 512

bf16 = ml_dtypes.bfloat16

_CACHE = {}


def _split_multi_waits(nc, mybir):
    """The TPB instruction encoding has a single wait slot; this walrus build
    rejects instructions carrying more than one sync wait. Hoist extra waits
    onto standalone EventSemaphore instructions on the same engine. Tile's
    schedule is a valid serialization (waits only reference earlier-ordered
    work on other streams), so blocking the issuing stream at the same point
    cannot deadlock."""
    SKIP = ("InstTriggerDma", "InstCollectiveCompute")
    for f in nc.m.functions:
        for blk in f.blocks:
            out = []
            changed = False
            for inst in blk.instructions:
                si = getattr(inst, "sync_info", None)
                ow = list(si.on_wait) if si is not None and si.on_wait else []
                if len(ow) > 1 and type(inst).__name__ not in SKIP:
                    for i, w in enumerate(ow[:-1]):
                        out.append(mybir.InstEventSemaphore(
                            name=f"{inst.name}_hw{i}",
                            engine=inst.engine,
                            sync_info=mybir.SyncInfo(on_wait=[w], on_update=[]),
                            bass_nofuse=True,
                        ))
                    inst.sync_info = mybir.SyncInfo(
                        on_wait=[ow[-1]],
                        on_update=list(si.on_update) if si.on_update else [],
                    )
                    changed = True
                out.append(inst)
            if changed:
                blk.instructions = out


def _build_bass():
    import concourse.bass as bass
    import concourse.mybir as mybir
    import concourse.tile as tile
    from contextlib import ExitStack

    dt = mybir.dt
    f32 = dt.float32
    bf = dt.bfloat16

    nc = bass.Bass()
    xT_d = nc.declare_dram_parameter("xT", [D, T], bf, isOutput=False)
    wqk_d = nc.declare_dram_parameter("wqk", [D, 2 * DG], bf, isOutput=False)
    wv_d = nc.declare_dram_parameter("wv", [D, DG], bf, isOutput=False)
    wo_d = nc.declare_dram_parameter("wo", [DG, D], bf, isOutput=False)
    mask_d = nc.declare_dram_parameter("masks", [128, 256], bf, isOutput=False)
    oh_d = nc.declare_dram_parameter("oh", [32, 32 * 64], bf, isOutput=False)
    out_d = nc.declare_dram_parameter("out", [T, D], f32, isOutput=True)

    with tile.TileContext(nc) as tc, ExitStack() as ctx:
        const = ctx.enter_context(tc.tile_pool(name="const", bufs=1))
        psum = ctx.enter_context(tc.tile_pool(name="psum", bufs=2, space="PSUM"))
        ptp = ctx.enter_context(tc.tile_pool(name="ptp", bufs=10))
        stp = ctx.enter_context(tc.tile_pool(name="stp", bufs=10))
        small = ctx.enter_context(tc.tile_pool(name="small", bufs=3))

        # ---- resident tensors --------------------------------------------
        xT_sb = const.tile([128, 8, T], bf)          # x[b].T   (feature-major)
        wqk_sb = const.tile([128, 8, 2 * DG], bf)    # W_qkv q|k columns
        wv_sb = const.tile([128, 8, DG], bf)         # W_qkv v columns
        wo_sb = const.tile([128, 4, D], bf)          # W_out rows for group
        qkT_sb = const.tile([128, 8, T], bf)         # [q^T | k^T]  (feature-major)
        vn_sb = const.tile([128, TCH, DG], bf)       # V natural per key chunk
        at_sb = const.tile([128, 4, T], bf)          # A^T (normalized attn out)
        mask_sb = const.tile([128, 256], bf)         # diagonal-block mask x2
        oh_sb = const.tile([32, 32 * 64], bf)        # one-hot lhsT for PE bcast
        ones_sb = const.tile([128, 1], bf)           # ones column for denoms

        # staged loads: earliest-needed first, split across the two queues
        for c in range(8):
            nc.sync.dma_start(out=xT_sb[:, c, 0:512], in_=xT_d[c * 128:(c + 1) * 128, 0:512])
            nc.gpsimd.dma_start(out=wv_sb[:, c, :], in_=wv_d[c * 128:(c + 1) * 128, :])
        for c in range(8):
            nc.gpsimd.dma_start(out=wqk_sb[:, c, :], in_=wqk_d[c * 128:(c + 1) * 128, :])
        for c in range(8):
            nc.sync.dma_start(out=xT_sb[:, c, 512:1024], in_=xT_d[c * 128:(c + 1) * 128, 512:1024])
        nc.sync.dma_start(out=mask_sb, in_=mask_d[:, :])
        nc.sync.dma_start(out=oh_sb, in_=oh_d[:, :])
        for c in range(8):
            nc.sync.dma_start(out=xT_sb[:, c, 1024:2048], in_=xT_d[c * 128:(c + 1) * 128, 1024:2048])
        for c in range(4):
            nc.gpsimd.dma_start(out=wo_sb[:, c, :], in_=wo_d[c * 128:(c + 1) * 128, :])
        nc.vector.memset(ones_sb, 1.0)

        def v_chunk_parts(tn):
            """Two ~850ns fill halves sharing one psum accumulation chain."""
            cell = {}

            def a():
                cell["pv"] = psum.tile([128, 512], f32, tag="work",
                                       name=f"pv{tn}")
                for k in range(4):
                    nc.tensor.matmul(
                        cell["pv"],
                        lhsT=xT_sb[:, k, tn * 128:(tn + 1) * 128],
                        rhs=wv_sb[:, k, :],
                        start=(k == 0), stop=False,
                    )

            def b():
                for k in range(4, 8):
                    nc.tensor.matmul(
                        cell["pv"],
                        lhsT=xT_sb[:, k, tn * 128:(tn + 1) * 128],
                        rhs=wv_sb[:, k, :],
                        start=False, stop=(k == 7),
                    )
                nc.vector.tensor_copy(out=vn_sb[:, tn, :], in_=cell["pv"])

            return [a, b]

        def v_chunk(tn):
            for u in v_chunk_parts(tn):
                u()

        def qk_unit_parts(m, n):
            cell = {}

            def a():
                cell["pq"] = psum.tile([128, 512], f32, tag="work",
                                       name=f"pq{m}_{n}")
                for k in range(4):
                    nc.tensor.matmul(
                        cell["pq"],
                        lhsT=wqk_sb[:, k, m * 128:(m + 1) * 128],
                        rhs=xT_sb[:, k, n * 512:(n + 1) * 512],
                        start=(k == 0), stop=False,
                    )

            def b():
                for k in range(4, 8):
                    nc.tensor.matmul(
                        cell["pq"],
                        lhsT=wqk_sb[:, k, m * 128:(m + 1) * 128],
                        rhs=xT_sb[:, k, n * 512:(n + 1) * 512],
                        start=False, stop=(k == 7),
                    )
                nc.vector.tensor_copy(
                    out=qkT_sb[:, m, n * 512:(n + 1) * 512], in_=cell["pq"])

            return [a, b]

        def qk_unit(m, n):
            for u in qk_unit_parts(m, n):
                u()

        colls = {}
        stages = {}

        def attn(qc, p, fill=None):
            """Scores + exp + AV + denominators for head pair p of query
            chunk qc. The inner loop emits only the score matmuls (row-tiled
            K=64 pairs) plus one ~850ns fill unit per key chunk, so ScalarE's
            exp chain paces the loop with the PE fully fed. AV (column-tiled
            pairs) and M=1 denominator matmuls are deferred into bursts of
            contiguous same-shape groups every BG key chunks — streaming-
            limited instead of paying a pipeline-switch per chunk."""
            BG = 8
            nfull = 4 * qc
            nkc = nfull + 4
            pav = psum.tile([128, 512], f32, tag="av", bufs=1, name=f"pav{qc}_{p}")
            den = psum.tile([128, 512], f32, tag="den", bufs=1, name=f"den{qc}_{p}")
            pts = [None] * nkc

            def width(kc):
                return 512 if kc < nfull else 512 - 128 * (kc - nfull)

            def burst(kcs):
                for kc in kcs:
                    w = width(kc)
                    qo = 512 - w
                    pt = pts[kc]
                    nc.tensor.matmul(
                        pav[0:64, qo:512],
                        lhsT=vn_sb[:, kc, 128 * p:128 * p + 64],
                        rhs=pt[:, 0:w],
                        start=(kc == 0), stop=(kc == nkc - 1),
                    )
                    nc.tensor.matmul(
                        pav[64:128, qo:512],
                        lhsT=vn_sb[:, kc, 128 * p + 64:128 * p + 128],
                        rhs=pt[:, w:2 * w],
                        start=(kc == 0), stop=(kc == nkc - 1),
                    )
                for kc in kcs:
                    w = width(kc)
                    qo = 512 - w
                    pt = pts[kc]
                    # odd full chunks accumulate at partitions 64/96 so the
                    # four M=1 matmuls run column-tiled; band chunks fold
                    # into the 0/32 chains (their first chunk covers the
                    # full query range, so no unwritten psum is ever read)
                    odd = kc < nfull and kc % 2 == 1
                    r = 64 if odd else 0
                    if odd:
                        st, sp = (kc == 1), (kc == nfull - 1)
                    else:
                        st, sp = (kc == 0), (kc == nkc - 1)
                    nc.tensor.matmul(
                        den[r:r + 1, qo:512], lhsT=ones_sb,
                        rhs=pt[:, 0:w], start=st, stop=sp,
                        tile_position=(0, r),
                    )
                    nc.tensor.matmul(
                        den[r + 32:r + 33, qo:512], lhsT=ones_sb,
                        rhs=pt[:, w:2 * w], start=st, stop=sp,
                        tile_position=(0, r + 32),
                    )

            for kc in range(nkc):
                w = width(kc)
                qo = 512 - w
                q0 = qc * 512 + qo
                q1 = (qc + 1) * 512
                ps = psum.tile([128, 1024], f32, tag="s", name=f"ps{qc}_{p}_{kc}")
                nc.tensor.matmul(
                    ps[:, 0:w],
                    lhsT=qkT_sb[0:64, 4 + p, kc * 128:(kc + 1) * 128],
                    rhs=qkT_sb[0:64, p, q0:q1], start=True, stop=True,
                )
                nc.tensor.matmul(
                    ps[:, 512:512 + w],
                    lhsT=qkT_sb[64:128, 4 + p, kc * 128:(kc + 1) * 128],
                    rhs=qkT_sb[64:128, p, q0:q1], start=True, stop=True,
                )
                if kc == BG:
                    burst(range(0, BG))
                elif fill and kc >= 1:
                    fill.pop(0)()  # PE fill work for exp-paced stalls
                pt = ptp.tile([128, 1024], bf, tag="pt", name=f"pt{qc}_{p}_{kc}")
                pts[kc] = pt
                nc.scalar.activation(
                    out=pt[:, 0:2 * w].rearrange("z (h c) -> z h c", h=2),
                    in_=ps.rearrange("z (h c) -> z h c", h=2)[:, :, 0:w],
                    func=mybir.ActivationFunctionType.Exp, scale=SCALE,
                )
                if kc >= nfull:
                    # mask the diagonal 128-query prefix of both heads
                    dv = pt[:, 0:2 * w].rearrange("z (h c) -> z h c", h=2)
                    nc.vector.tensor_mul(
                        out=dv[:, :, 0:128], in0=dv[:, :, 0:128],
                        in1=mask_sb.rearrange("z (h c) -> z h c", h=2),
                    )
            burst(range(BG if nkc > BG else 0, nkc))

            stage = stp.tile([128, 512], bf, tag="stage", bufs=12,
                             name=f"st{qc}_{p}")
            nc.vector.tensor_copy(out=stage, in_=pav)
            stages[(qc, p)] = stage
            if p == 0:
                colls[qc] = (
                    stp.tile([32, 128], f32, tag="collev", bufs=2,
                             name=f"collev{qc}"),
                    stp.tile([32, 128], f32, tag="collod", bufs=2,
                             name=f"collod{qc}") if qc > 0 else None,
                )
            collev, collod = colls[qc]
            # DMA cannot read PSUM: bounce den through SBUF (rows 0/32 hold
            # the even-parity sums, 64/96 the odd ones; qc 0 has no odd rows)
            den_sb = stp.tile([128, 512], f32, tag="densb", bufs=4,
                              name=f"densb{qc}_{p}")
            if qc > 0:
                nc.vector.tensor_copy(out=den_sb, in_=den)
            else:
                nc.vector.tensor_copy(out=den_sb[0:64, :], in_=den[0:64, :])
            for i, r in enumerate((0, 32)):
                nc.gpsimd.dma_start(
                    out=collev[8 * p + 4 * i:8 * p + 4 * i + 4, :],
                    in_=den_sb[r:r + 1, :].rearrange("o (a c) -> o a c", c=128),
                )
            if qc > 0:
                for i, r in enumerate((64, 96)):
                    nc.gpsimd.dma_start(
                        out=collod[8 * p + 4 * i:8 * p + 4 * i + 4, :],
                        in_=den_sb[r:r + 1, :].rearrange("o (a c) -> o a c", c=128),
                    )

        def divisions(qc):
            collev, collod = colls[qc]
            if collod is not None:
                coll = stp.tile([32, 128], f32, tag="coll", bufs=2)
                nc.vector.tensor_add(out=coll, in0=collev, in1=collod)
            else:
                coll = collev
            rcol = stp.tile([32, 128], f32, tag="rcol", bufs=2)
            nc.vector.reciprocal(rcol, coll)
            rcolb = stp.tile([32, 128], bf, tag="rcolb", bufs=2)
            nc.vector.tensor_copy(out=rcolb, in_=rcol)
            for pp in range(4):
                # replicate each head's per-query reciprocal across the 64
                # feature partitions via one-hot stationary matmuls; the two
                # heads' copies run column-tiled at array columns 0/64
                prb = psum.tile([128, 512], f32, tag="work", name=f"prb{qc}_{pp}")
                for hp in range(2):
                    for a in range(4):
                        j = 8 * pp + 4 * hp + a
                        nc.tensor.matmul(
                            prb[64 * hp:64 * hp + 64, a * 128:(a + 1) * 128],
                            lhsT=oh_sb[:, j * 64:(j + 1) * 64],
                            rhs=rcolb[:, :],
                            start=True, stop=True,
                        )
                nc.vector.tensor_mul(
                    out=at_sb[:, pp, qc * 512:(qc + 1) * 512],
                    in0=stages[(qc, pp)], in1=prb,
                )

        def outproj_unit(qj, dn):
            def emit():
                po = psum.tile([128, 512], f32, tag="work",
                               name=f"po{qj}_{dn}")
                for kc in range(4):
                    nc.tensor.matmul(
                        po,
                        lhsT=at_sb[:, kc, qj * 128:(qj + 1) * 128],
                        rhs=wo_sb[:, kc, dn * 512:(dn + 1) * 512],
                        start=(kc == 0), stop=(kc == 3),
                    )
                ost = small.tile([128, 512], f32, tag="ost")
                nc.vector.tensor_copy(out=ost, in_=po)
                nc.sync.dma_start(
                    out=out_d[qj * 128:(qj + 1) * 128,
                              dn * 512:(dn + 1) * 512],
                    in_=ost,
                )
            return emit

        def outproj_units(qc):
            return [outproj_unit(qj, dn)
                    for qj in range(4 * qc, 4 * qc + 4) for dn in range(2)]

        def outproj(qc):
            for u in outproj_units(qc):
                u()

        # One global PE fill queue: the attention loops pop one ~850ns unit
        # per key chunk to cover the exp-paced PE slack. Units are ordered so
        # every dependency (vn chunks, qk features, divisions) is produced
        # before its first consumer.
        fill = []

        def qk_whole(m, n):
            return [lambda: qk_unit(m, n)]

        def v_whole(tn):
            return v_chunk_parts(tn)

        for tn in range(4):
            v_chunk(tn)
        qk_unit(0, 0)
        qk_unit(4, 0)
        for p in range(4):
            if p > 0:
                qk_unit(p, 0)
                qk_unit(4 + p, 0)
            attn(0, p, fill)
            if p == 0:
                for tn in range(4, 8):
                    v_chunk(tn)
        for p in range(4):
            qk_unit(p, 1)
            qk_unit(4 + p, 1)
            attn(1, p, fill)
        divisions(0)
        for tn in range(8, 12):
            fill += v_whole(tn)
        fill += outproj_units(0)
        for p in range(4):
            qk_unit(p, 2)
            qk_unit(4 + p, 2)
            attn(2, p, fill)
            if p == 2:
                divisions(1)
        for tn in range(12, 16):
            fill += v_whole(tn)
        fill += outproj_units(1)
        for p in range(4):
            qk_unit(p, 3)
            qk_unit(4 + p, 3)
            attn(3, p, fill)
            if p == 2:
                divisions(2)
                fill += outproj_units(2)
        divisions(3)
        while fill:
            fill.pop(0)()
        outproj(3)

    _split_multi_waits(nc, mybir)
    return nc


def _make_masks():
    kl = np.arange(128)[:, None]
    ql = np.arange(128)[None, :]
    m = (ql >= kl).astype(np.float32)
    return np.concatenate([m, m], axis=1).astype(bf16)  # [128, 256]


def _make_in_maps(x, W_qkv, W_out):
    masks = _make_masks()
    # oh[k, 64*j + m] = (k == j): one-hot stationary used to replicate
    # reciprocal rows across partitions on the TensorEngine
    oh = np.zeros((32, 32, 64), np.float32)
    for j in range(32):
        oh[j, j, :] = 1.0
    oh = oh.reshape(32, 32 * 64).astype(bf16)
    in_maps = []
    for c in range(NCORES):
        b, g = divmod(c, 2)
        xT = np.ascontiguousarray(x[b].T).astype(bf16)
        wq = W_qkv[:, g * DG:(g + 1) * DG]
        wk = W_qkv[:, D + g * DG:D + (g + 1) * DG]
        wv = W_qkv[:, 2 * D + g * DG:2 * D + (g + 1) * DG]
        wqk = np.concatenate([wq, wk], axis=1).astype(bf16)
        wo = W_out[g * DG:(g + 1) * DG, :].astype(bf16)
        in_maps.append({
            "xT": xT,
            "wqk": wqk,
            "wv": np.ascontiguousarray(wv).astype(bf16),
            "wo": np.ascontiguousarray(wo),
            "masks": masks,
            "oh": oh,
        })
    return in_maps


def _np_fallback(x, W_qkv, b_qkv, W_out, b_out):
    out = np.empty((B, T, D), np.float32)
    qkv = x.reshape(B * T, D) @ W_qkv + b_qkv
    q, k, v = np.split(qkv.reshape(B, T, 3 * D), 3, axis=-1)

    def heads(z):
        return z.reshape(B, T, H, HD).transpose(0, 2, 1, 3)

    q, k, v = heads(q), heads(k), heads(v)
    causal = np.tril(np.ones((T, T), dtype=bool))
    acc = np.empty((B, H, T, HD), np.float32)
    for bi in range(B):
        for h in range(H):
            s = (q[bi, h] @ k[bi, h].T) * np.float32(SCALE)
            s = np.where(causal, s, -np.inf)
            s -= s.max(axis=-1, keepdims=True)
            p = np.exp(s)
            p /= p.sum(axis=-1, keepdims=True)
            acc[bi, h] = p @ v[bi, h]
    a = acc.transpose(0, 2, 1, 3).reshape(B, T, D)
    for bi in range(B):
        out[bi] = a[bi] @ W_out + b_out
    return out


def run(x, W_qkv, b_qkv, W_out, b_out, trace=False, trace_kwargs=None):
    from concourse import bass_utils

    x = np.asarray(x, np.float32)
    W_qkv = np.asarray(W_qkv, np.float32)
    b_qkv = np.asarray(b_qkv, np.float32)
    W_out = np.asarray(W_out, np.float32)
    b_out = np.asarray(b_out, np.float32)

    # the on-device kernel assumes b_qkv == 0 (true for this problem
    # family; b_out is applied on the host). Fall back if not.
    if np.any(b_qkv):
        return _np_fallback(x, W_qkv, b_qkv, W_out, b_out), None

    if "nc" not in _CACHE:
        _CACHE["nc"] = _build_bass()
    nc = _CACHE["nc"]

    in_maps = _make_in_maps(x, W_qkv, W_out)
    kw = dict(trace=trace)
    if trace_kwargs:
        kw.update(trace_kwargs)
    res = bass_utils.run_bass_kernel_spmd(nc, in_maps, list(range(NCORES)), **kw)

    out = np.empty((B, T, D), np.float32)
    for b in range(B):
        out[b] = (np.asarray(res.results[2 * b]["out"], np.float32)
                  + np.asarray(res.results[2 * b + 1]["out"], np.float32)
                  + b_out)
    return out, res


def kernel(x, W_qkv, b_qkv, W_out, b_out):
    out, _ = run(x, W_qkv, b_qkv, W_out, b_out, trace=False)
    return out


# revision 41
# speedup vs baseline: 1.0065x; 1.0065x over previous
"""Causal self-attention on 8 TRN2 NeuronCores.

Sharding: core c = (batch b = c // 2, head-group g = c % 2).
Each core handles one batch and 8 of the 16 heads:
  - QKV projection for its 512 q/k/v feature slices (transposed layout)
  - causal attention for its 8 heads
  - partial output projection (its 512 rows of W_out)
Host sums the two partials per batch and adds b_out.

All TensorE matmuls run in bf16; softmax runs in f32 (exp on ScalarE,
normalization via M=1 ones-matmul denominators + VectorE reciprocal).

Key structure:
  - Scores matmuls have K=64 contraction; even/odd heads of a pair sit at
    SBUF partitions 0-63 / 64-127 and run concurrently in the top/bottom
    halves of the PE array (row tiling).
  - Causal band: for the diagonal key chunks only the live query suffix
    (width W = 512-128*l) is computed, exp'd and AV'd.
  - AV is column-tiled: the two heads' V tiles sit in array columns 0-63 /
    64-127 and stream their probability tiles concurrently (2x vs M=65).
  - Softmax denominators are M=1 ones-stationary matmuls into four PSUM
    partitions (0/32/64/96 by key-chunk parity), also running column-tiled.
"""

import numpy as np
import ml_dtypes

B, T, D, H = 4, 2048, 1024, 16
HG = 2            # head groups (tensor-parallel factor)
HL = H // HG      # 8 heads per core
HD = D // H       # 64
DG = HL * HD      # 512 features per group
SCALE = 1.0 / float(np.sqrt(HD))
NCORES = 8
TCH = T // 128    # 16 key chunks of 128
NQC = T // 512    # 4 query chunks of 512

bf16 = ml_dtypes.bfloat16

_CACHE = {}


def _split_multi_waits(nc, mybir):
    """The TPB instruction encoding has a single wait slot; this walrus build
    rejects instructions carrying more than one sync wait. Hoist extra waits
    onto standalone EventSemaphore instructions on the same engine. Tile's
    schedule is a valid serialization (waits only reference earlier-ordered
    work on other streams), so blocking the issuing stream at the same point
    cannot deadlock."""
    SKIP = ("InstTriggerDma", "InstCollectiveCompute")
    for f in nc.m.functions:
        for blk in f.blocks:
            out = []
            changed = False
            for inst in blk.instructions:
                si = getattr(inst, "sync_info", None)
                ow = list(si.on_wait) if si is not None and si.on_wait else []
                if len(ow) > 1 and type(inst).__name__ not in SKIP:
                    for i, w in enumerate(ow[:-1]):
                        out.append(mybir.InstEventSemaphore(
                            name=f"{inst.name}_hw{i}",
                            engine=inst.engine,
                            sync_info=mybir.SyncInfo(on_wait=[w], on_update=[]),
                            bass_nofuse=True,
                        ))
                    inst.sync_info = mybir.SyncInfo(
                        on_wait=[ow[-1]],
                        on_update=list(si.on_update) if si.on_update else [],
                    )
                    changed = True
                out.append(inst)
            if changed:
                blk.instructions = out


def _build_bass():
    import concourse.bass as bass
    import concourse.mybir as mybir
    import concourse.tile as tile
    from contextlib import ExitStack

    dt = mybir.dt
    f32 = dt.float32
    bf = dt.bfloat16

    nc = bass.Bass()
    xT_d = nc.declare_dram_parameter("xT", [D, T], bf, isOutput=False)
    wqk_d = nc.declare_dram_parameter("wqk", [D, 2 * DG], bf, isOutput=False)
    wv_d = nc.declare_dram_parameter("wv", [D, DG], bf, isOutput=False)
    wo_d = nc.declare_dram_parameter("wo", [DG, D], bf, isOutput=False)
    mask_d = nc.declare_dram_parameter("masks", [128, 256], bf, isOutput=False)
    oh_d = nc.declare_dram_parameter("oh", [32, 32 * 64], bf, isOutput=False)
    out_d = nc.declare_dram_parameter("out", [T, D], f32, isOutput=True)

    with tile.TileContext(nc) as tc, ExitStack() as ctx:
        const = ctx.enter_context(tc.tile_pool(name="const", bufs=1))
        psum = ctx.enter_context(tc.tile_pool(name="psum", bufs=2, space="PSUM"))
        ptp = ctx.enter_context(tc.tile_pool(name="ptp", bufs=17))
        stp = ctx.enter_context(tc.tile_pool(name="stp", bufs=10))
        small = ctx.enter_context(tc.tile_pool(name="small", bufs=3))

        # ---- resident tensors --------------------------------------------
        xT_sb = const.tile([128, 8, T], bf)          # x[b].T   (feature-major)
        wqk_sb = const.tile([128, 8, 2 * DG], bf)    # W_qkv q|k columns
        wv_sb = const.tile([128, 8, DG], bf)         # W_qkv v columns
        wo_sb = const.tile([128, 4, D], bf)          # W_out rows for group
        qkT_sb = const.tile([128, 8, T], bf)         # [q^T | k^T]  (feature-major)
        vn_sb = const.tile([128, TCH, DG], bf)       # V natural per key chunk
        at_sb = const.tile([128, 4, T], bf)          # A^T (normalized attn out)
        mask_sb = const.tile([128, 256], bf)         # diagonal-block mask x2
        oh_sb = const.tile([32, 32 * 64], bf)        # one-hot lhsT for PE bcast
        ones_sb = const.tile([128, 1], bf)           # ones column for denoms

        # staged loads: earliest-needed first, split across the two queues
        for c in range(8):
            nc.sync.dma_start(out=xT_sb[:, c, 0:512], in_=xT_d[c * 128:(c + 1) * 128, 0:512])
            nc.gpsimd.dma_start(out=wv_sb[:, c, :], in_=wv_d[c * 128:(c + 1) * 128, :])
        for c in range(8):
            nc.gpsimd.dma_start(out=wqk_sb[:, c, :], in_=wqk_d[c * 128:(c + 1) * 128, :])
        for c in range(8):
            nc.sync.dma_start(out=xT_sb[:, c, 512:1024], in_=xT_d[c * 128:(c + 1) * 128, 512:1024])
        nc.sync.dma_start(out=mask_sb, in_=mask_d[:, :])
        nc.sync.dma_start(out=oh_sb, in_=oh_d[:, :])
        for c in range(8):
            nc.sync.dma_start(out=xT_sb[:, c, 1024:2048], in_=xT_d[c * 128:(c + 1) * 128, 1024:2048])
        for c in range(4):
            nc.gpsimd.dma_start(out=wo_sb[:, c, :], in_=wo_d[c * 128:(c + 1) * 128, :])
        nc.vector.memset(ones_sb, 1.0)

        def v_chunk_parts(tn):
            """Two ~850ns fill halves sharing one psum accumulation chain."""
            cell = {}

            def a():
                cell["pv"] = psum.tile([128, 512], f32, tag="work",
                                       name=f"pv{tn}")
                for k in range(4):
                    nc.tensor.matmul(
                        cell["pv"],
                        lhsT=xT_sb[:, k, tn * 128:(tn + 1) * 128],
                        rhs=wv_sb[:, k, :],
                        start=(k == 0), stop=False,
                    )

            def b():
                for k in range(4, 8):
                    nc.tensor.matmul(
                        cell["pv"],
                        lhsT=xT_sb[:, k, tn * 128:(tn + 1) * 128],
                        rhs=wv_sb[:, k, :],
                        start=False, stop=(k == 7),
                    )
                nc.vector.tensor_copy(out=vn_sb[:, tn, :], in_=cell["pv"])

            return [a, b]

        def v_chunk(tn):
            for u in v_chunk_parts(tn):
                u()

        def qk_unit_parts(m, n):
            cell = {}

            def a():
                cell["pq"] = psum.tile([128, 512], f32, tag="work",
                                       name=f"pq{m}_{n}")
                for k in range(4):
                    nc.tensor.matmul(
                        cell["pq"],
                        lhsT=wqk_sb[:, k, m * 128:(m + 1) * 128],
                        rhs=xT_sb[:, k, n * 512:(n + 1) * 512],
                        start=(k == 0), stop=False,
                    )

            def b():
                for k in range(4, 8):
                    nc.tensor.matmul(
                        cell["pq"],
                        lhsT=wqk_sb[:, k, m * 128:(m + 1) * 128],
                        rhs=xT_sb[:, k, n * 512:(n + 1) * 512],
                        start=False, stop=(k == 7),
                    )
                nc.vector.tensor_copy(
                    out=qkT_sb[:, m, n * 512:(n + 1) * 512], in_=cell["pq"])

            return [a, b]

        def qk_unit(m, n):
            for u in qk_unit_parts(m, n):
                u()

        colls = {}
        stages = {}

        def attn(qc, p, fill=None):
            """Scores + exp + AV + denominators for head pair p of query
            chunk qc. The inner loop emits only the score matmuls (row-tiled
            K=64 pairs) plus one ~850ns fill unit per key chunk, so ScalarE's
            exp chain paces the loop with the PE fully fed. AV (column-tiled
            pairs) and M=1 denominator matmuls are deferred into bursts of
            contiguous same-shape groups every BG key chunks — streaming-
            limited instead of paying a pipeline-switch per chunk."""
            BG = 16
            nfull = 4 * qc
            nkc = nfull + 4
            pav = psum.tile([128, 512], f32, tag="av", bufs=1, name=f"pav{qc}_{p}")
            den = psum.tile([128, 512], f32, tag="den", bufs=1, name=f"den{qc}_{p}")
            pts = [None] * nkc

            def width(kc):
                return 512 if kc < nfull else 512 - 128 * (kc - nfull)

            def burst(kcs):
                for kc in kcs:
                    w = width(kc)
                    qo = 512 - w
                    pt = pts[kc]
                    nc.tensor.matmul(
                        pav[0:64, qo:512],
                        lhsT=vn_sb[:, kc, 128 * p:128 * p + 64],
                        rhs=pt[:, 0:w],
                        start=(kc == 0), stop=(kc == nkc - 1),
                    )
                    nc.tensor.matmul(
                        pav[64:128, qo:512],
                        lhsT=vn_sb[:, kc, 128 * p + 64:128 * p + 128],
                        rhs=pt[:, w:2 * w],
                        start=(kc == 0), stop=(kc == nkc - 1),
                    )
                for kc in kcs:
                    w = width(kc)
                    qo = 512 - w
                    pt = pts[kc]
                    # odd full chunks accumulate at partitions 64/96 so the
                    # four M=1 matmuls run column-tiled; band chunks fold
                    # into the 0/32 chains (their first chunk covers the
                    # full query range, so no unwritten psum is ever read)
                    odd = kc < nfull and kc % 2 == 1
                    r = 64 if odd else 0
                    if odd:
                        st, sp = (kc == 1), (kc == nfull - 1)
                    else:
                        st, sp = (kc == 0), (kc == nkc - 1)
                    nc.tensor.matmul(
                        den[r:r + 1, qo:512], lhsT=ones_sb,
                        rhs=pt[:, 0:w], start=st, stop=sp,
                        tile_position=(0, r),
                    )
                    nc.tensor.matmul(
                        den[r + 32:r + 33, qo:512], lhsT=ones_sb,
                        rhs=pt[:, w:2 * w], start=st, stop=sp,
                        tile_position=(0, r + 32),
                    )

            for kc in range(nkc):
                w = width(kc)
                qo = 512 - w
                q0 = qc * 512 + qo
                q1 = (qc + 1) * 512
                ps = psum.tile([128, 1024], f32, tag="s", name=f"ps{qc}_{p}_{kc}")
                nc.tensor.matmul(
                    ps[:, 0:w],
                    lhsT=qkT_sb[0:64, 4 + p, kc * 128:(kc + 1) * 128],
                    rhs=qkT_sb[0:64, p, q0:q1], start=True, stop=True,
                )
                nc.tensor.matmul(
                    ps[:, 512:512 + w],
                    lhsT=qkT_sb[64:128, 4 + p, kc * 128:(kc + 1) * 128],
                    rhs=qkT_sb[64:128, p, q0:q1], start=True, stop=True,
                )
                if kc == BG:
                    burst(range(0, BG))
                elif fill and kc >= 1:
                    fill.pop(0)()  # PE fill work for exp-paced stalls
                pt = ptp.tile([128, 1024], bf, tag="pt", name=f"pt{qc}_{p}_{kc}")
                pts[kc] = pt
                nc.scalar.activation(
                    out=pt[:, 0:2 * w].rearrange("z (h c) -> z h c", h=2),
                    in_=ps.rearrange("z (h c) -> z h c", h=2)[:, :, 0:w],
                    func=mybir.ActivationFunctionType.Exp, scale=SCALE,
                )
                if kc >= nfull:
                    # mask the diagonal 128-query prefix of both heads
                    dv = pt[:, 0:2 * w].rearrange("z (h c) -> z h c", h=2)
                    nc.vector.tensor_mul(
                        out=dv[:, :, 0:128], in0=dv[:, :, 0:128],
                        in1=mask_sb.rearrange("z (h c) -> z h c", h=2),
                    )
            burst(range(BG if nkc > BG else 0, nkc))

            stage = stp.tile([128, 512], bf, tag="stage", bufs=12,
                             name=f"st{qc}_{p}")
            nc.vector.tensor_copy(out=stage, in_=pav)
            stages[(qc, p)] = stage
            if p == 0:
                colls[qc] = (
                    stp.tile([32, 128], f32, tag="collev", bufs=2,
                             name=f"collev{qc}"),
                    stp.tile([32, 128], f32, tag="collod", bufs=2,
                             name=f"collod{qc}") if qc > 0 else None,
                )
            collev, collod = colls[qc]
            # DMA cannot read PSUM: bounce den through SBUF (rows 0/32 hold
            # the even-parity sums, 64/96 the odd ones; qc 0 has no odd rows)
            den_sb = stp.tile([128, 512], f32, tag="densb", bufs=4,
                              name=f"densb{qc}_{p}")
            if qc > 0:
                nc.vector.tensor_copy(out=den_sb, in_=den)
            else:
                nc.vector.tensor_copy(out=den_sb[0:64, :], in_=den[0:64, :])
            for i, r in enumerate((0, 32)):
                nc.gpsimd.dma_start(
                    out=collev[8 * p + 4 * i:8 * p + 4 * i + 4, :],
                    in_=den_sb[r:r + 1, :].rearrange("o (a c) -> o a c", c=128),
                )
            if qc > 0:
                for i, r in enumerate((64, 96)):
                    nc.gpsimd.dma_start(
                        out=collod[8 * p + 4 * i:8 * p + 4 * i + 4, :],
                        in_=den_sb[r:r + 1, :].rearrange("o (a c) -> o a c", c=128),
                    )

        def divisions(qc):
            collev, collod = colls[qc]
            if collod is not None:
                coll = stp.tile([32, 128], f32, tag="coll", bufs=2)
                nc.vector.tensor_add(out=coll, in0=collev, in1=collod)
            else:
                coll = collev
            rcol = stp.tile([32, 128], f32, tag="rcol", bufs=2)
            nc.vector.reciprocal(rcol, coll)
            rcolb = stp.tile([32, 128], bf, tag="rcolb", bufs=2)
            nc.vector.tensor_copy(out=rcolb, in_=rcol)
            for pp in range(4):
                # replicate each head's per-query reciprocal across the 64
                # feature partitions via one-hot stationary matmuls; the two
                # heads' copies run column-tiled at array columns 0/64
                prb = psum.tile([128, 512], f32, tag="work", name=f"prb{qc}_{pp}")
                for hp in range(2):
                    for a in range(4):
                        j = 8 * pp + 4 * hp + a
                        nc.tensor.matmul(
                            prb[64 * hp:64 * hp + 64, a * 128:(a + 1) * 128],
                            lhsT=oh_sb[:, j * 64:(j + 1) * 64],
                            rhs=rcolb[:, :],
                            start=True, stop=True,
                        )
                nc.vector.tensor_mul(
                    out=at_sb[:, pp, qc * 512:(qc + 1) * 512],
                    in0=stages[(qc, pp)], in1=prb,
                )

        def outproj_unit(qj, dn):
            def emit():
                po = psum.tile([128, 512], f32, tag="work",
                               name=f"po{qj}_{dn}")
                for kc in range(4):
                    nc.tensor.matmul(
                        po,
                        lhsT=at_sb[:, kc, qj * 128:(qj + 1) * 128],
                        rhs=wo_sb[:, kc, dn * 512:(dn + 1) * 512],
                        start=(kc == 0), stop=(kc == 3),
                    )
                ost = small.tile([128, 512], f32, tag="ost")
                nc.vector.tensor_copy(out=ost, in_=po)
                nc.sync.dma_start(
                    out=out_d[qj * 128:(qj + 1) * 128,
                              dn * 512:(dn + 1) * 512],
                    in_=ost,
                )
            return emit

        def outproj_units(qc):
            return [outproj_unit(qj, dn)
                    for qj in range(4 * qc, 4 * qc + 4) for dn in range(2)]

        def outproj(qc):
            for u in outproj_units(qc):
                u()

        # One global PE fill queue: the attention loops pop one ~850ns unit
        # per key chunk to cover the exp-paced PE slack. Units are ordered so
        # every dependency (vn chunks, qk features, divisions) is produced
        # before its first consumer.
        fill = []

        def qk_whole(m, n):
            return [lambda: qk_unit(m, n)]

        def v_whole(tn):
            return v_chunk_parts(tn)

        for tn in range(4):
            v_chunk(tn)
        qk_unit(0, 0)
        qk_unit(4, 0)
        for p in range(4):
            if p > 0:
                qk_unit(p, 0)
                qk_unit(4 + p, 0)
            attn(0, p, fill)
            if p == 0:
                for tn in range(4, 8):
                    v_chunk(tn)
        for p in range(4):
            qk_unit(p, 1)
            qk_unit(4 + p, 1)
            attn(1, p, fill)
        divisions(0)
        for tn in range(8, 12):
            fill += v_whole(tn)
        fill += outproj_units(0)
        for p in range(4):
            qk_unit(p, 2)
            qk_unit(4 + p, 2)
            attn(2, p, fill)
            if p == 2:
                divisions(1)
        for tn in range(12, 16):
            fill += v_whole(tn)
        fill += outproj_units(1)
        for p in range(4):
            qk_unit(p, 3)
            qk_unit(4 + p, 3)
            attn(3, p, fill)
            if p == 2:
                divisions(2)
                fill += outproj_units(2)
        while fill:
            fill.pop(0)()
        divisions(3)
        outproj(3)

    _split_multi_waits(nc, mybir)
    return nc


def _make_masks():
    kl = np.arange(128)[:, None]
    ql = np.arange(128)[None, :]
    m = (ql >= kl).astype(np.float32)
    return np.concatenate([m, m], axis=1).astype(bf16)  # [128, 256]


def _make_in_maps(x, W_qkv, W_out):
    masks = _make_masks()
    # oh[k, 64*j + m] = (k == j): one-hot stationary used to replicate
    # reciprocal rows across partitions on the TensorEngine
    oh = np.zeros((32, 32, 64), np.float32)
    for j in range(32):
        oh[j, j, :] = 1.0
    oh = oh.reshape(32, 32 * 64).astype(bf16)
    in_maps = []
    for c in range(NCORES):
        b, g = divmod(c, 2)
        xT = np.ascontiguousarray(x[b].T).astype(bf16)
        wq = W_qkv[:, g * DG:(g + 1) * DG]
        wk = W_qkv[:, D + g * DG:D + (g + 1) * DG]
        wv = W_qkv[:, 2 * D + g * DG:2 * D + (g + 1) * DG]
        wqk = np.concatenate([wq, wk], axis=1).astype(bf16)
        wo = W_out[g * DG:(g + 1) * DG, :].astype(bf16)
        in_maps.append({
            "xT": xT,
            "wqk": wqk,
            "wv": np.ascontiguousarray(wv).astype(bf16),
            "wo": np.ascontiguousarray(wo),
            "masks": masks,
            "oh": oh,
        })
    return in_maps


def _np_fallback(x, W_qkv, b_qkv, W_out, b_out):
    out = np.empty((B, T, D), np.float32)
    qkv = x.reshape(B * T, D) @ W_qkv + b_qkv
    q, k, v = np.split(qkv.reshape(B, T, 3 * D), 3, axis=-1)

    def heads(z):
        return z.reshape(B, T, H, HD).transpose(0, 2, 1, 3)

    q, k, v = heads(q), heads(k), heads(v)
    causal = np.tril(np.ones((T, T), dtype=bool))
    acc = np.empty((B, H, T, HD), np.float32)
    for bi in range(B):
        for h in range(H):
            s = (q[bi, h] @ k[bi, h].T) * np.float32(SCALE)
            s = np.where(causal, s, -np.inf)
            s -= s.max(axis=-1, keepdims=True)
            p = np.exp(s)
            p /= p.sum(axis=-1, keepdims=True)
            acc[bi, h] = p @ v[bi, h]
    a = acc.transpose(0, 2, 1, 3).reshape(B, T, D)
    for bi in range(B):
        out[bi] = a[bi] @ W_out + b_out
    return out


def run(x, W_qkv, b_qkv, W_out, b_out, trace=False, trace_kwargs=None):
    from concourse import bass_utils

    x = np.asarray(x, np.float32)
    W_qkv = np.asarray(W_qkv, np.float32)
    b_qkv = np.asarray(b_qkv, np.float32)
    W_out = np.asarray(W_out, np.float32)
    b_out = np.asarray(b_out, np.float32)

    # the on-device kernel assumes b_qkv == 0 (true for this problem
    # family; b_out is applied on the host). Fall back if not.
    if np.any(b_qkv):
        return _np_fallback(x, W_qkv, b_qkv, W_out, b_out), None

    if "nc" not in _CACHE:
        _CACHE["nc"] = _build_bass()
    nc = _CACHE["nc"]

    in_maps = _make_in_maps(x, W_qkv, W_out)
    kw = dict(trace=trace)
    if trace_kwargs:
        kw.update(trace_kwargs)
    res = bass_utils.run_bass_kernel_spmd(nc, in_maps, list(range(NCORES)), **kw)

    out = np.empty((B, T, D), np.float32)
    for b in range(B):
        out[b] = (np.asarray(res.results[2 * b]["out"], np.float32)
                  + np.asarray(res.results[2 * b + 1]["out"], np.float32)
                  + b_out)
    return out, res


def kernel(x, W_qkv, b_qkv, W_out, b_out):
    out, _ = run(x, W_qkv, b_qkv, W_out, b_out, trace=False)
    return out


# revision 42
# speedup vs baseline: 1.0123x; 1.0057x over previous
"""Causal self-attention on 8 TRN2 NeuronCores.

Sharding: core c = (batch b = c // 2, head-group g = c % 2).
Each core handles one batch and 8 of the 16 heads:
  - QKV projection for its 512 q/k/v feature slices (transposed layout)
  - causal attention for its 8 heads
  - partial output projection (its 512 rows of W_out)
Host sums the two partials per batch and adds b_out.

All TensorE matmuls run in bf16; softmax runs in f32 (exp on ScalarE,
normalization via M=1 ones-matmul denominators + VectorE reciprocal).

Key structure:
  - Scores matmuls have K=64 contraction; even/odd heads of a pair sit at
    SBUF partitions 0-63 / 64-127 and run concurrently in the top/bottom
    halves of the PE array (row tiling).
  - Causal band: for the diagonal key chunks only the live query suffix
    (width W = 512-128*l) is computed, exp'd and AV'd.
  - AV is column-tiled: the two heads' V tiles sit in array columns 0-63 /
    64-127 and stream their probability tiles concurrently (2x vs M=65).
  - Softmax denominators are M=1 ones-stationary matmuls into four PSUM
    partitions (0/32/64/96 by key-chunk parity), also running column-tiled.
"""

import numpy as np
import ml_dtypes

B, T, D, H = 4, 2048, 1024, 16
HG = 2            # head groups (tensor-parallel factor)
HL = H // HG      # 8 heads per core
HD = D // H       # 64
DG = HL * HD      # 512 features per group
SCALE = 1.0 / float(np.sqrt(HD))
NCORES = 8
TCH = T // 128    # 16 key chunks of 128
NQC = T // 512    # 4 query chunks of 512

bf16 = ml_dtypes.bfloat16

_CACHE = {}


def _split_multi_waits(nc, mybir):
    """The TPB instruction encoding has a single wait slot; this walrus build
    rejects instructions carrying more than one sync wait. Hoist extra waits
    onto standalone EventSemaphore instructions on the same engine. Tile's
    schedule is a valid serialization (waits only reference earlier-ordered
    work on other streams), so blocking the issuing stream at the same point
    cannot deadlock."""
    SKIP = ("InstTriggerDma", "InstCollectiveCompute")
    for f in nc.m.functions:
        for blk in f.blocks:
            out = []
            changed = False
            for inst in blk.instructions:
                si = getattr(inst, "sync_info", None)
                ow = list(si.on_wait) if si is not None and si.on_wait else []
                if len(ow) > 1 and type(inst).__name__ not in SKIP:
                    for i, w in enumerate(ow[:-1]):
                        out.append(mybir.InstEventSemaphore(
                            name=f"{inst.name}_hw{i}",
                            engine=inst.engine,
                            sync_info=mybir.SyncInfo(on_wait=[w], on_update=[]),
                            bass_nofuse=True,
                        ))
                    inst.sync_info = mybir.SyncInfo(
                        on_wait=[ow[-1]],
                        on_update=list(si.on_update) if si.on_update else [],
                    )
                    changed = True
                out.append(inst)
            if changed:
                blk.instructions = out


def _build_bass():
    import concourse.bass as bass
    import concourse.mybir as mybir
    import concourse.tile as tile
    from contextlib import ExitStack

    dt = mybir.dt
    f32 = dt.float32
    bf = dt.bfloat16

    nc = bass.Bass()
    xT_d = nc.declare_dram_parameter("xT", [D, T], bf, isOutput=False)
    wqk_d = nc.declare_dram_parameter("wqk", [D, 2 * DG], bf, isOutput=False)
    wv_d = nc.declare_dram_parameter("wv", [D, DG], bf, isOutput=False)
    wo_d = nc.declare_dram_parameter("wo", [DG, D], bf, isOutput=False)
    mask_d = nc.declare_dram_parameter("masks", [128, 256], bf, isOutput=False)
    oh_d = nc.declare_dram_parameter("oh", [32, 32 * 64], bf, isOutput=False)
    out_d = nc.declare_dram_parameter("out", [T, D], f32, isOutput=True)

    with tile.TileContext(nc) as tc, ExitStack() as ctx:
        const = ctx.enter_context(tc.tile_pool(name="const", bufs=1))
        psum = ctx.enter_context(tc.tile_pool(name="psum", bufs=2, space="PSUM"))
        ptp = ctx.enter_context(tc.tile_pool(name="ptp", bufs=10))
        stp = ctx.enter_context(tc.tile_pool(name="stp", bufs=10))
        small = ctx.enter_context(tc.tile_pool(name="small", bufs=3))

        # ---- resident tensors --------------------------------------------
        xT_sb = const.tile([128, 8, T], bf)          # x[b].T   (feature-major)
        wqk_sb = const.tile([128, 8, 2 * DG], bf)    # W_qkv q|k columns
        wv_sb = const.tile([128, 8, DG], bf)         # W_qkv v columns
        wo_sb = const.tile([128, 4, D], bf)          # W_out rows for group
        qkT_sb = const.tile([128, 8, T], bf)         # [q^T | k^T]  (feature-major)
        vn_sb = const.tile([128, TCH, DG], bf)       # V natural per key chunk
        at_sb = const.tile([128, 4, T], bf)          # A^T (normalized attn out)
        mask_sb = const.tile([128, 256], bf)         # diagonal-block mask x2
        oh_sb = const.tile([32, 32 * 64], bf)        # one-hot lhsT for PE bcast
        ones_sb = const.tile([128, 1], bf)           # ones column for denoms

        # staged loads: earliest-needed first, split across the two queues
        for c in range(8):
            nc.sync.dma_start(out=xT_sb[:, c, 0:512], in_=xT_d[c * 128:(c + 1) * 128, 0:512])
            nc.gpsimd.dma_start(out=wv_sb[:, c, :], in_=wv_d[c * 128:(c + 1) * 128, :])
        for c in range(8):
            nc.gpsimd.dma_start(out=wqk_sb[:, c, :], in_=wqk_d[c * 128:(c + 1) * 128, :])
        for c in range(8):
            nc.sync.dma_start(out=xT_sb[:, c, 512:1024], in_=xT_d[c * 128:(c + 1) * 128, 512:1024])
        nc.sync.dma_start(out=mask_sb, in_=mask_d[:, :])
        nc.sync.dma_start(out=oh_sb, in_=oh_d[:, :])
        for c in range(8):
            nc.sync.dma_start(out=xT_sb[:, c, 1024:2048], in_=xT_d[c * 128:(c + 1) * 128, 1024:2048])
        for c in range(4):
            nc.gpsimd.dma_start(out=wo_sb[:, c, :], in_=wo_d[c * 128:(c + 1) * 128, :])
        nc.vector.memset(ones_sb, 1.0)

        def v_chunk_parts(tn):
            """Two ~850ns fill halves sharing one psum accumulation chain."""
            cell = {}

            def a():
                cell["pv"] = psum.tile([128, 512], f32, tag="work",
                                       name=f"pv{tn}")
                for k in range(4):
                    nc.tensor.matmul(
                        cell["pv"],
                        lhsT=xT_sb[:, k, tn * 128:(tn + 1) * 128],
                        rhs=wv_sb[:, k, :],
                        start=(k == 0), stop=False,
                    )

            def b():
                for k in range(4, 8):
                    nc.tensor.matmul(
                        cell["pv"],
                        lhsT=xT_sb[:, k, tn * 128:(tn + 1) * 128],
                        rhs=wv_sb[:, k, :],
                        start=False, stop=(k == 7),
                    )
                nc.vector.tensor_copy(out=vn_sb[:, tn, :], in_=cell["pv"])

            return [a, b]

        def v_chunk(tn):
            for u in v_chunk_parts(tn):
                u()

        def qk_unit_parts(m, n):
            cell = {}

            def a():
                cell["pq"] = psum.tile([128, 512], f32, tag="work",
                                       name=f"pq{m}_{n}")
                for k in range(4):
                    nc.tensor.matmul(
                        cell["pq"],
                        lhsT=wqk_sb[:, k, m * 128:(m + 1) * 128],
                        rhs=xT_sb[:, k, n * 512:(n + 1) * 512],
                        start=(k == 0), stop=False,
                    )

            def b():
                for k in range(4, 8):
                    nc.tensor.matmul(
                        cell["pq"],
                        lhsT=wqk_sb[:, k, m * 128:(m + 1) * 128],
                        rhs=xT_sb[:, k, n * 512:(n + 1) * 512],
                        start=False, stop=(k == 7),
                    )
                nc.vector.tensor_copy(
                    out=qkT_sb[:, m, n * 512:(n + 1) * 512], in_=cell["pq"])

            return [a, b]

        def qk_unit(m, n):
            for u in qk_unit_parts(m, n):
                u()

        colls = {}
        stages = {}

        def attn(qc, p, fill=None):
            """Scores + exp + AV + denominators for head pair p of query
            chunk qc. The inner loop emits only the score matmuls (row-tiled
            K=64 pairs) plus one ~850ns fill unit per key chunk, so ScalarE's
            exp chain paces the loop with the PE fully fed. AV (column-tiled
            pairs) and M=1 denominator matmuls are deferred into bursts of
            contiguous same-shape groups every BG key chunks — streaming-
            limited instead of paying a pipeline-switch per chunk."""
            BG = 8
            nfull = 4 * qc
            nkc = nfull + 4
            pav = psum.tile([128, 512], f32, tag="av", bufs=1, name=f"pav{qc}_{p}")
            den = psum.tile([128, 512], f32, tag="den", bufs=1, name=f"den{qc}_{p}")
            pts = [None] * nkc

            def width(kc):
                return 512 if kc < nfull else 512 - 128 * (kc - nfull)

            def burst(kcs):
                for kc in kcs:
                    w = width(kc)
                    qo = 512 - w
                    pt = pts[kc]
                    nc.tensor.matmul(
                        pav[0:64, qo:512],
                        lhsT=vn_sb[:, kc, 128 * p:128 * p + 64],
                        rhs=pt[:, 0:w],
                        start=(kc == 0), stop=(kc == nkc - 1),
                    )
                    nc.tensor.matmul(
                        pav[64:128, qo:512],
                        lhsT=vn_sb[:, kc, 128 * p + 64:128 * p + 128],
                        rhs=pt[:, w:2 * w],
                        start=(kc == 0), stop=(kc == nkc - 1),
                    )
                for kc in kcs:
                    w = width(kc)
                    qo = 512 - w
                    pt = pts[kc]
                    # odd full chunks accumulate at partitions 64/96 so the
                    # four M=1 matmuls run column-tiled; band chunks fold
                    # into the 0/32 chains (their first chunk covers the
                    # full query range, so no unwritten psum is ever read)
                    odd = kc < nfull and kc % 2 == 1
                    r = 64 if odd else 0
                    if odd:
                        st, sp = (kc == 1), (kc == nfull - 1)
                    else:
                        st, sp = (kc == 0), (kc == nkc - 1)
                    nc.tensor.matmul(
                        den[r:r + 1, qo:512], lhsT=ones_sb,
                        rhs=pt[:, 0:w], start=st, stop=sp,
                        tile_position=(0, r),
                    )
                    nc.tensor.matmul(
                        den[r + 32:r + 33, qo:512], lhsT=ones_sb,
                        rhs=pt[:, w:2 * w], start=st, stop=sp,
                        tile_position=(0, r + 32),
                    )

            for kc in range(nkc):
                w = width(kc)
                qo = 512 - w
                q0 = qc * 512 + qo
                q1 = (qc + 1) * 512
                ps = psum.tile([128, 1024], f32, tag="s", name=f"ps{qc}_{p}_{kc}")
                nc.tensor.matmul(
                    ps[:, 0:w],
                    lhsT=qkT_sb[0:64, 4 + p, kc * 128:(kc + 1) * 128],
                    rhs=qkT_sb[0:64, p, q0:q1], start=True, stop=True,
                )
                nc.tensor.matmul(
                    ps[:, 512:512 + w],
                    lhsT=qkT_sb[64:128, 4 + p, kc * 128:(kc + 1) * 128],
                    rhs=qkT_sb[64:128, p, q0:q1], start=True, stop=True,
                )
                if kc == BG:
                    burst(range(0, BG))
                elif fill and kc >= 1:
                    fill.pop(0)()  # PE fill work for exp-paced stalls
                pt = ptp.tile([128, 1024], bf, tag="pt", name=f"pt{qc}_{p}_{kc}")
                pts[kc] = pt
                nc.scalar.activation(
                    out=pt[:, 0:2 * w].rearrange("z (h c) -> z h c", h=2),
                    in_=ps.rearrange("z (h c) -> z h c", h=2)[:, :, 0:w],
                    func=mybir.ActivationFunctionType.Exp, scale=SCALE,
                )
                if kc >= nfull:
                    # mask the diagonal 128-query prefix of both heads
                    dv = pt[:, 0:2 * w].rearrange("z (h c) -> z h c", h=2)
                    nc.vector.tensor_mul(
                        out=dv[:, :, 0:128], in0=dv[:, :, 0:128],
                        in1=mask_sb.rearrange("z (h c) -> z h c", h=2),
                    )
            burst(range(BG if nkc > BG else 0, nkc))

            stage = stp.tile([128, 512], bf, tag="stage", bufs=12,
                             name=f"st{qc}_{p}")
            nc.vector.tensor_copy(out=stage, in_=pav)
            stages[(qc, p)] = stage
            if p == 0:
                colls[qc] = (
                    stp.tile([32, 128], f32, tag="collev", bufs=2,
                             name=f"collev{qc}"),
                    stp.tile([32, 128], f32, tag="collod", bufs=2,
                             name=f"collod{qc}") if qc > 0 else None,
                )
            collev, collod = colls[qc]
            # DMA cannot read PSUM: bounce den through SBUF (rows 0/32 hold
            # the even-parity sums, 64/96 the odd ones; qc 0 has no odd rows)
            den_sb = stp.tile([128, 512], f32, tag="densb", bufs=4,
                              name=f"densb{qc}_{p}")
            if qc > 0:
                nc.vector.tensor_copy(out=den_sb, in_=den)
            else:
                nc.vector.tensor_copy(out=den_sb[0:64, :], in_=den[0:64, :])
            for i, r in enumerate((0, 32)):
                nc.gpsimd.dma_start(
                    out=collev[8 * p + 4 * i:8 * p + 4 * i + 4, :],
                    in_=den_sb[r:r + 1, :].rearrange("o (a c) -> o a c", c=128),
                )
            if qc > 0:
                for i, r in enumerate((64, 96)):
                    nc.gpsimd.dma_start(
                        out=collod[8 * p + 4 * i:8 * p + 4 * i + 4, :],
                        in_=den_sb[r:r + 1, :].rearrange("o (a c) -> o a c", c=128),
                    )

        def divisions(qc):
            collev, collod = colls[qc]
            if collod is not None:
                coll = stp.tile([32, 128], f32, tag="coll", bufs=2)
                nc.vector.tensor_add(out=coll, in0=collev, in1=collod)
            else:
                coll = collev
            rcol = stp.tile([32, 128], f32, tag="rcol", bufs=2)
            nc.vector.reciprocal(rcol, coll)
            rcolb = stp.tile([32, 128], bf, tag="rcolb", bufs=2)
            nc.vector.tensor_copy(out=rcolb, in_=rcol)
            for pp in range(4):
                # replicate each head's per-query reciprocal across the 64
                # feature partitions via one-hot stationary matmuls; the two
                # heads' copies run column-tiled at array columns 0/64
                prb = psum.tile([128, 512], f32, tag="work", name=f"prb{qc}_{pp}")
                for hp in range(2):
                    for a in range(4):
                        j = 8 * pp + 4 * hp + a
                        nc.tensor.matmul(
                            prb[64 * hp:64 * hp + 64, a * 128:(a + 1) * 128],
                            lhsT=oh_sb[:, j * 64:(j + 1) * 64],
                            rhs=rcolb[:, :],
                            start=True, stop=True,
                        )
                nc.vector.tensor_mul(
                    out=at_sb[:, pp, qc * 512:(qc + 1) * 512],
                    in0=stages[(qc, pp)], in1=prb,
                )

        def outproj_unit(qj, dn):
            def emit():
                po = psum.tile([128, 512], f32, tag="work",
                               name=f"po{qj}_{dn}")
                for kc in range(4):
                    nc.tensor.matmul(
                        po,
                        lhsT=at_sb[:, kc, qj * 128:(qj + 1) * 128],
                        rhs=wo_sb[:, kc, dn * 512:(dn + 1) * 512],
                        start=(kc == 0), stop=(kc == 3),
                    )
                ost = small.tile([128, 512], f32, tag="ost")
                nc.vector.tensor_copy(out=ost, in_=po)
                nc.sync.dma_start(
                    out=out_d[qj * 128:(qj + 1) * 128,
                              dn * 512:(dn + 1) * 512],
                    in_=ost,
                )
            return emit

        def outproj_units(qc):
            return [outproj_unit(qj, dn)
                    for qj in range(4 * qc, 4 * qc + 4) for dn in range(2)]

        def outproj(qc):
            for u in outproj_units(qc):
                u()

        # One global PE fill queue: the attention loops pop one ~850ns unit
        # per key chunk to cover the exp-paced PE slack. Units are ordered so
        # every dependency (vn chunks, qk features, divisions) is produced
        # before its first consumer.
        fill = []

        def qk_whole(m, n):
            return [lambda: qk_unit(m, n)]

        def v_whole(tn):
            return v_chunk_parts(tn)

        for tn in range(4):
            v_chunk(tn)
        qk_unit(0, 0)
        qk_unit(4, 0)
        for p in range(4):
            if p > 0:
                qk_unit(p, 0)
                qk_unit(4 + p, 0)
            attn(0, p, fill)
            if p == 0:
                for tn in range(4, 8):
                    v_chunk(tn)
        for p in range(4):
            qk_unit(p, 1)
            qk_unit(4 + p, 1)
            attn(1, p, fill)
        divisions(0)
        for tn in range(8, 12):
            fill += v_whole(tn)
        fill += outproj_units(0)
        for p in range(4):
            qk_unit(p, 2)
            qk_unit(4 + p, 2)
            attn(2, p, fill)
            if p == 2:
                divisions(1)
        for tn in range(12, 16):
            fill += v_whole(tn)
        fill += outproj_units(1)
        for p in range(4):
            qk_unit(p, 3)
            qk_unit(4 + p, 3)
            attn(3, p, fill)
            if p == 2:
                divisions(2)
                fill += outproj_units(2)
        while fill:
            fill.pop(0)()
        divisions(3)
        outproj(3)

    _split_multi_waits(nc, mybir)
    return nc


def _make_masks():
    kl = np.arange(128)[:, None]
    ql = np.arange(128)[None, :]
    m = (ql >= kl).astype(np.float32)
    return np.concatenate([m, m], axis=1).astype(bf16)  # [128, 256]


def _make_in_maps(x, W_qkv, W_out):
    masks = _make_masks()
    # oh[k, 64*j + m] = (k == j): one-hot stationary used to replicate
    # reciprocal rows across partitions on the TensorEngine
    oh = np.zeros((32, 32, 64), np.float32)
    for j in range(32):
        oh[j, j, :] = 1.0
    oh = oh.reshape(32, 32 * 64).astype(bf16)
    in_maps = []
    for c in range(NCORES):
        b, g = divmod(c, 2)
        xT = np.ascontiguousarray(x[b].T).astype(bf16)
        wq = W_qkv[:, g * DG:(g + 1) * DG]
        wk = W_qkv[:, D + g * DG:D + (g + 1) * DG]
        wv = W_qkv[:, 2 * D + g * DG:2 * D + (g + 1) * DG]
        wqk = np.concatenate([wq, wk], axis=1).astype(bf16)
        wo = W_out[g * DG:(g + 1) * DG, :].astype(bf16)
        in_maps.append({
            "xT": xT,
            "wqk": wqk,
            "wv": np.ascontiguousarray(wv).astype(bf16),
            "wo": np.ascontiguousarray(wo),
            "masks": masks,
            "oh": oh,
        })
    return in_maps


def _np_fallback(x, W_qkv, b_qkv, W_out, b_out):
    out = np.empty((B, T, D), np.float32)
    qkv = x.reshape(B * T, D) @ W_qkv + b_qkv
    q, k, v = np.split(qkv.reshape(B, T, 3 * D), 3, axis=-1)

    def heads(z):
        return z.reshape(B, T, H, HD).transpose(0, 2, 1, 3)

    q, k, v = heads(q), heads(k), heads(v)
    causal = np.tril(np.ones((T, T), dtype=bool))
    acc = np.empty((B, H, T, HD), np.float32)
    for bi in range(B):
        for h in range(H):
            s = (q[bi, h] @ k[bi, h].T) * np.float32(SCALE)
            s = np.where(causal, s, -np.inf)
            s -= s.max(axis=-1, keepdims=True)
            p = np.exp(s)
            p /= p.sum(axis=-1, keepdims=True)
            acc[bi, h] = p @ v[bi, h]
    a = acc.transpose(0, 2, 1, 3).reshape(B, T, D)
    for bi in range(B):
        out[bi] = a[bi] @ W_out + b_out
    return out


def run(x, W_qkv, b_qkv, W_out, b_out, trace=False, trace_kwargs=None):
    from concourse import bass_utils

    x = np.asarray(x, np.float32)
    W_qkv = np.asarray(W_qkv, np.float32)
    b_qkv = np.asarray(b_qkv, np.float32)
    W_out = np.asarray(W_out, np.float32)
    b_out = np.asarray(b_out, np.float32)

    # the on-device kernel assumes b_qkv == 0 (true for this problem
    # family; b_out is applied on the host). Fall back if not.
    if np.any(b_qkv):
        return _np_fallback(x, W_qkv, b_qkv, W_out, b_out), None

    if "nc" not in _CACHE:
        _CACHE["nc"] = _build_bass()
    nc = _CACHE["nc"]

    in_maps = _make_in_maps(x, W_qkv, W_out)
    kw = dict(trace=trace)
    if trace_kwargs:
        kw.update(trace_kwargs)
    res = bass_utils.run_bass_kernel_spmd(nc, in_maps, list(range(NCORES)), **kw)

    out = np.empty((B, T, D), np.float32)
    for b in range(B):
        out[b] = (np.asarray(res.results[2 * b]["out"], np.float32)
                  + np.asarray(res.results[2 * b + 1]["out"], np.float32)
                  + b_out)
    return out, res


def kernel(x, W_qkv, b_qkv, W_out, b_out):
    out, _ = run(x, W_qkv, b_qkv, W_out, b_out, trace=False)
    return out
